# revision 8
# baseline (speedup 1.0000x reference)
"""CoAttention kernel v5 for 8 Trainium2 NeuronCores.

Problem: S, D: [8, 2048, 1024] f32, one batch per core.
  G = D @ S^T                      [2048, 2048]
  co_D = D + rowsoftmax(G) @ S
  co_S = S + rowsoftmax(G^T) @ D

Design notes (v5):
 - S^T is built by PE transposes interleaved with the DMA-bound S load
   stream in the prologue (xbar-transposing it there floods the shared
   DMA-semaphore pool and serializes the loads -- 100us regression).
 - D^T tiles (dt) and W^T tiles (wt) come from DMA-xbar transposes
   (dma_start(transpose=True), ~1.3/1.9us each) issued in the main loop
   where DMA traffic is sparse; wt_i is issued one iteration before its
   consumer so xbar latency hides under G matmuls.
 - A dummy-MM warmup burst plus one keep-warm dummy per S block holds
   the PE HAM clock gate at 2.4 GHz through the prologue (PE transposes
   do not count as HAM-busy; v2 ran its first 27us at 1.2 GHz).
 - Pools are scoped: the 5-deep S stage pool closes after the prologue
   (5-deep is needed to hide per-hop semaphore latency; 2-deep ran the
   load stream at 1/3 speed), freeing SBUF for the wt/outp pools.
 - colsum finalize (16 PE f32 transposes) folded into iters 14/15;
   phase C emits co_S per half, stores split across two queues.
 - Residual adds use the resident f16 S_nat/D_nat (~2e-4 extra rel
   err), so phase C reloads nothing from DRAM.

Softmax trick (v2): shift-invariance with constant SHIFT; shared
W = exp(G - SHIFT) bf16 serves both directions:
  co_D[l] = D[l] + (W @ S)[l] / rowsum_l(W)
  co_S[m] = S[m] + (W^T @ D)[m] / colsum_m(W)
"""

import numpy as np

P = 128
T = 2048
DH = 1024
LT = T // P     # 16 token blocks per side
KD = DH // P    # 8 contraction blocks
NTILE = 512
NCH = T // NTILE  # 4 chunks of the m axis
SHIFT = 100.0

DEFAULTS = dict(
    warm_mms=8,
    keep_warm=1,
    stageS_bufs=5,
    stageD_bufs=2,
    gpsum_bufs=2,
    opsum_bufs=1,
    dtp_bufs=4,
    wtp_bufs=3,
    outp_bufs=2,
)

_CACHE = {}


def _build_nc(**overrides):
    import concourse.mybir as mybir
    import concourse.tile as tile
    from concourse import bacc
    from concourse.masks import make_identity

    p = dict(DEFAULTS)
    p.update(overrides)

    dt = mybir.dt
    f32, f16, bf16 = dt.float32, dt.float16, dt.bfloat16
    AX = mybir.AxisListType.X
    EXP = mybir.ActivationFunctionType.Exp
    MULT = mybir.AluOpType.mult
    ADD = mybir.AluOpType.add

    nc = bacc.Bacc("TRN2", target_bir_lowering=False, debug=False)

    S_ap = nc.dram_tensor("S", [T, DH], f32, kind="ExternalInput").ap()
    D_ap = nc.dram_tensor("D", [T, DH], f32, kind="ExternalInput").ap()
    coD_ap = nc.dram_tensor("co_D", [T, DH], f32, kind="ExternalOutput").ap()
    coS_ap = nc.dram_tensor("co_S", [T, DH], f32, kind="ExternalOutput").ap()

    with tile.TileContext(nc) as tc:
        with (
            tc.tile_pool(name="consts", bufs=1) as consts,
            tc.tile_pool(name="big", bufs=1) as big,
            tc.tile_pool(name="stageD", bufs=p["stageD_bufs"]) as stageD,
            tc.tile_pool(name="rspp", bufs=3) as rspp,
            tc.tile_pool(name="small", bufs=4) as small,
        ):
            ident_f32 = consts.tile([P, P], f32)
            make_identity(nc, ident_f32[:])
            ident_f16 = consts.tile([P, P], f16)
            make_identity(nc, ident_f16[:])
            nbias = consts.tile([P, 1], f32)
            nc.vector.memset(nbias[:], -SHIFT)
            warm_src = consts.tile([P, NTILE], f16)
            nc.vector.memset(warm_src[:], 0.0)

            S_nat = big.tile([P, LT, DH], f16)     # [m%128, (mblk, d)]
            S_T = big.tile([P, KD, T], f16)        # [d%128, (dblk, m)]
            D_nat = big.tile([P, LT, DH], f16)     # [l%128, (lblk, d)]
            W = big.tile([P, LT, T], bf16)         # [l%128, (lblk, m)]
            S1 = big.tile([P, T], f32)             # partial colsums
            nc.vector.memset(S1[:], 0.0)

            gps_ctx = tc.tile_pool(name="gpsum", bufs=p["gpsum_bufs"], space="PSUM")
            gpsum = gps_ctx.__enter__()
            ops_ctx = tc.tile_pool(name="opsum", bufs=p["opsum_bufs"], space="PSUM")
            opsum = ops_ctx.__enter__()
            dtp_ctx = tc.tile_pool(name="dtp", bufs=p["dtp_bufs"])
            dtp = dtp_ctx.__enter__()
            wps_ctx = tc.tile_pool(name="warmps", bufs=1, space="PSUM")
            warmps = wps_ctx.__enter__()
            tps_ctx = tc.tile_pool(name="tps", bufs=2, space="PSUM")
            tps = tps_ctx.__enter__()
            stS_ctx = tc.tile_pool(name="stageS", bufs=p["stageS_bufs"])
            stageS = stS_ctx.__enter__()

            wps = warmps.tile([P, NTILE], f32)

            def _warm(n):
                for _ in range(n):
                    nc.tensor.matmul(wps[:], warm_src[:, 0:P], warm_src[:],
                                     start=True, stop=True)

            def _load_d(i):
                t_ = stageD.tile([P, DH], f32, tag="ldd", name="std")
                nc.gpsimd.dma_start(t_[:], D_ap[i * P:(i + 1) * P, :])
                return t_

            def _conv_d(i, t_):
                nc.gpsimd.tensor_copy(D_nat[:, i, :], t_[:])

            def _mk_dt(i):
                dti = dtp.tile([P, KD, P], f16, tag="dt", name="dt")
                nc.scalar.dma_start(dti[:], D_nat[:, i, :], transpose=True)
                return dti

            def _g_chunk(i, mc, dt_i, rsp):
                gp = gpsum.tile([P, NTILE], f32, tag="g")
                for k in range(KD):
                    nc.tensor.matmul(
                        gp[:],
                        dt_i[:, k, :],
                        S_T[:, k, mc * NTILE:(mc + 1) * NTILE],
                        start=(k == 0),
                        stop=(k == KD - 1),
                    )
                nc.scalar.activation(
                    W[:, i, mc * NTILE:(mc + 1) * NTILE], gp[:], EXP,
                    bias=nbias[:], scale=1.0,
                    accum_out=rsp[:, mc:mc + 1],
                )
                nc.vector.tensor_add(
                    S1[:, mc * NTILE:(mc + 1) * NTILE],
                    S1[:, mc * NTILE:(mc + 1) * NTILE],
                    W[:, i, mc * NTILE:(mc + 1) * NTILE],
                )

            # ---- Prologue ----
            _warm(p["warm_mms"])

            dts = {}
            std_tiles = {0: _load_d(0), 1: _load_d(1)}
            rsps = {0: rspp.tile([P, NCH], f32, tag="rsp", name="rsp0"),
                    1: rspp.tile([P, NCH], f32, tag="rsp", name="rsp1")}

            st_tiles = {}
            for j in range(p["stageS_bufs"]):
                st_tiles[j] = stageS.tile([P, DH], f32, tag="ld", name="st")
                nc.sync.dma_start(st_tiles[j][:], S_ap[j * P:(j + 1) * P, :])

            for j in range(LT):
                if j + p["stageS_bufs"] < LT:
                    k = j + p["stageS_bufs"]
                    st_tiles[k] = stageS.tile([P, DH], f32, tag="ld", name="st")
                    nc.sync.dma_start(st_tiles[k][:], S_ap[k * P:(k + 1) * P, :])
                stj = st_tiles.pop(j)
                nc.vector.tensor_copy(S_nat[:, j, :], stj[:])
                # S^T for block j via PE transposes (overlaps DMA-bound loads)
                for g in range(2):
                    pt = tps.tile([P, 4, P], f16, tag="tp")
                    for k4 in range(4):
                        k = g * 4 + k4
                        nc.tensor.transpose(
                            pt[:, k4, :], S_nat[:, j, k * P:(k + 1) * P],
                            ident_f16[:],
                        )
                    nc.vector.tensor_copy(
                        S_T[:, g * 4:(g + 1) * 4, j * P:(j + 1) * P], pt[:]
                    )
                _warm(p["keep_warm"])

                # D pipeline milestones + early G chunks
                if j == 1:
                    _conv_d(0, std_tiles.pop(0))
                    std_tiles[2] = _load_d(2)
                elif j == 2:
                    _conv_d(1, std_tiles.pop(1))
                    std_tiles[3] = _load_d(3)
                    dts[0] = _mk_dt(0)
                    dts[1] = _mk_dt(1)
                elif j == 3:
                    _conv_d(2, std_tiles.pop(2))
                elif j == 4:
                    _conv_d(3, std_tiles.pop(3))
                    dts[2] = _mk_dt(2)
                    dts[3] = _mk_dt(3)
                    _g_chunk(0, 0, dts[0], rsps[0])
                    _g_chunk(1, 0, dts[1], rsps[1])
                elif j == 8:
                    _g_chunk(0, 1, dts[0], rsps[0])
                    _g_chunk(1, 1, dts[1], rsps[1])
                elif j == 12:
                    _g_chunk(0, 2, dts[0], rsps[0])
                    _g_chunk(1, 2, dts[1], rsps[1])
            _g_chunk(0, 3, dts[0], rsps[0])
            _g_chunk(1, 3, dts[1], rsps[1])
            dts.pop(0)
            dts.pop(1)

            stS_ctx.__exit__(None, None, None)
            tps_ctx.__exit__(None, None, None)
            wps_ctx.__exit__(None, None, None)

            # ---- Main loop: iter i runs G(i+2) and O_D(i) ----
            outp_ctx = tc.tile_pool(name="outp", bufs=p["outp_bufs"])
            outp = outp_ctx.__enter__()
            wtp_ctx = tc.tile_pool(name="wtp", bufs=p["wtp_bufs"])
            wtp = wtp_ctx.__enter__()
            tpsC_ctx = tc.tile_pool(name="tpsC", bufs=2, space="PSUM")
            tpsC = tpsC_ctx.__enter__()

            def _mk_wt(i):
                wti = wtp.tile([P, LT, P], bf16, tag="wt", name="wt")
                nc.scalar.dma_start(wti[:], W[:, i, :], transpose=True)
                return wti

            # W rows 0/1 are complete; transpose them ahead of the loop
            wts = {0: _mk_wt(0), 1: _mk_wt(1)}

            rcs = None
            cs_p = None
            for i in range(LT):
                # D pipeline: convert block i+3, dt-xbar i+3, load block i+4
                if i + 3 < LT and i + 3 >= 4:
                    _conv_d(i + 3, std_tiles.pop(i + 3))
                    dts[i + 3] = _mk_dt(i + 3)
                if i + 4 < LT:
                    std_tiles[i + 4] = _load_d(i + 4)

                if i + 2 < LT:
                    rsps[i + 2] = rspp.tile([P, NCH], f32, tag="rsp",
                                            name="rspn")
                    dt_i = dts.pop(i + 2)
                    for mc in range(NCH):
                        _g_chunk(i + 2, mc, dt_i, rsps[i + 2])
                    # W row i+2 complete -> issue its W^T xbar now; it
                    # finishes under the next iteration's G matmuls
                    wts[i + 2] = _mk_wt(i + 2)

                # colsum finalize folded into the last two iterations
                if i >= LT - 2:
                    if i == LT - 2:
                        cs_p = small.tile([P, LT], f32, tag="csp")
                    base = (i - (LT - 2)) * 8
                    for jj in range(base, base + 8):
                        ptc = tpsC.tile([P, P], f32, tag="tc")
                        nc.tensor.transpose(
                            ptc[:], S1[:, jj * P:(jj + 1) * P], ident_f32[:]
                        )
                        nc.vector.reduce_sum(cs_p[:, jj:jj + 1], ptc[:], axis=AX)
                    if i == LT - 1:
                        rcs = small.tile([P, LT], f32, tag="rcs")
                        nc.vector.reciprocal(rcs[:], cs_p[:])

                rsp = rsps.pop(i)
                rs = small.tile([P, 1], f32, tag="rs")
                nc.vector.reduce_sum(rs[:], rsp[:], axis=AX)
                rrs = small.tile([P, 1], f32, tag="rrs")
                nc.vector.reciprocal(rrs[:], rs[:])

                wt = wts.pop(i)
                ps = opsum.tile([P, DH], f32, tag="od")
                for kb in range(LT):
                    for n in range(DH // NTILE):
                        nc.tensor.matmul(
                            ps[:, n * NTILE:(n + 1) * NTILE],
                            wt[:, kb, :],
                            S_nat[:, kb, n * NTILE:(n + 1) * NTILE],
                            start=(kb == 0),
                            stop=(kb == LT - 1),
                        )
                o = outp.tile([P, DH], f32, tag="o")
                nc.vector.scalar_tensor_tensor(
                    o[:], ps[:], rrs[:], D_nat[:, i, :], MULT, ADD
                )
                nc.sync.dma_start(coD_ap[i * P:(i + 1) * P, :], o[:])

            tpsC_ctx.__exit__(None, None, None)
            wtp_ctx.__exit__(None, None, None)

            # ---- Phase C: O_S = W.T @ D_nat, emit co_S ----
            opc_ctx = tc.tile_pool(name="opc", bufs=2, space="PSUM")
            opc = opc_ctx.__enter__()
            for j in range(LT):
                ps = opc.tile([P, DH], f32, tag="os")
                o_j = outp.tile([P, DH], f32, tag="o", name="o_j")
                for n in range(2):
                    for lb in range(LT):
                        nc.tensor.matmul(
                            ps[:, n * NTILE:(n + 1) * NTILE],
                            W[:, lb, j * P:(j + 1) * P],
                            D_nat[:, lb, n * NTILE:(n + 1) * NTILE],
                            start=(lb == 0),
                            stop=(lb == LT - 1),
                        )
                    # half n complete: emit it while the other half runs
                    hs = slice(n * NTILE, (n + 1) * NTILE)
                    nc.vector.scalar_tensor_tensor(
                        o_j[:, hs], ps[:, hs], rcs[:, j:j + 1],
                        S_nat[:, j, hs], MULT, ADD,
                    )
                    q = nc.gpsimd if j % 2 == 0 else nc.sync
                    q.dma_start(coS_ap[j * P:(j + 1) * P, hs], o_j[:, hs])
            opc_ctx.__exit__(None, None, None)
            outp_ctx.__exit__(None, None, None)
            dtp_ctx.__exit__(None, None, None)
            ops_ctx.__exit__(None, None, None)
            gps_ctx.__exit__(None, None, None)

    nc.compile()
    return nc


def _get_nc():
    if "nc" not in _CACHE:
        import json as _json
        import os as _o
        ov = _json.loads(_o.environ.get("KOPTS", "{}"))
        _CACHE["nc"] = _build_nc(**ov)
    return _CACHE["nc"]


def kernel(S, D):
    from concourse.bass_utils import run_bass_kernel_spmd

    S = np.ascontiguousarray(np.asarray(S, dtype=np.float32))
    D = np.ascontiguousarray(np.asarray(D, dtype=np.float32))
    B = S.shape[0]
    assert S.shape == (B, T, DH) and D.shape == (B, T, DH) and B == 8

    nc = _get_nc()
    in_maps = [{"S": S[b], "D": D[b]} for b in range(B)]
    res = run_bass_kernel_spmd(nc, in_maps, core_ids=list(range(B)))
    co_D = np.stack([res.results[b]["co_D"] for b in range(B)])
    co_S = np.stack([res.results[b]["co_S"] for b in range(B)])
    return (co_D, co_S)


# revision 9
# speedup vs baseline: 1.1825x; 1.1825x over previous
"""CoAttention kernel v5 for 8 Trainium2 NeuronCores.

Problem: S, D: [8, 2048, 1024] f32, one batch per core.
  G = D @ S^T                      [2048, 2048]
  co_D = D + rowsoftmax(G) @ S
  co_S = S + rowsoftmax(G^T) @ D

Design notes (v5):
 - S^T is built by PE transposes interleaved with the DMA-bound S load
   stream in the prologue (xbar-transposing it there floods the shared
   DMA-semaphore pool and serializes the loads -- 100us regression).
 - D^T tiles (dt) and W^T tiles (wt) come from DMA-xbar transposes
   (dma_start(transpose=True), ~1.3/1.9us each) issued in the main loop
   where DMA traffic is sparse; wt_i is issued one iteration before its
   consumer so xbar latency hides under G matmuls.
 - A dummy-MM warmup burst plus one keep-warm dummy per S block holds
   the PE HAM clock gate at 2.4 GHz through the prologue (PE transposes
   do not count as HAM-busy; v2 ran its first 27us at 1.2 GHz).
 - Pools are scoped: the 5-deep S stage pool closes after the prologue
   (5-deep is needed to hide per-hop semaphore latency; 2-deep ran the
   load stream at 1/3 speed), freeing SBUF for the wt/outp pools.
 - colsum finalize (16 PE f32 transposes) folded into iters 14/15;
   phase C emits co_S per half, stores split across two queues.
 - Residual adds use the resident f16 S_nat/D_nat (~2e-4 extra rel
   err), so phase C reloads nothing from DRAM.

Softmax trick (v2): shift-invariance with constant SHIFT; shared
W = exp(G - SHIFT) bf16 serves both directions:
  co_D[l] = D[l] + (W @ S)[l] / rowsum_l(W)
  co_S[m] = S[m] + (W^T @ D)[m] / colsum_m(W)
"""

import numpy as np

P = 128
T = 2048
DH = 1024
LT = T // P     # 16 token blocks per side
KD = DH // P    # 8 contraction blocks
NTILE = 512
NCH = T // NTILE  # 4 chunks of the m axis
SHIFT = 100.0

DEFAULTS = dict(
    warm_mms=40,
    keep_warm=2,
    stageS_bufs=2,
    stageD_bufs=2,
    gpsum_bufs=2,
    opsum_bufs=1,
    dtp_bufs=4,
    wtp_bufs=3,
    outp_bufs=2,
)

_CACHE = {}


def _build_nc(**overrides):
    import concourse.mybir as mybir
    import concourse.tile as tile
    from concourse import bacc
    from concourse.masks import make_identity

    p = dict(DEFAULTS)
    p.update(overrides)

    dt = mybir.dt
    f32, f16, bf16 = dt.float32, dt.float16, dt.bfloat16
    AX = mybir.AxisListType.X
    EXP = mybir.ActivationFunctionType.Exp
    MULT = mybir.AluOpType.mult
    ADD = mybir.AluOpType.add

    nc = bacc.Bacc("TRN2", target_bir_lowering=False, debug=False)

    S_ap = nc.dram_tensor("S", [T, DH], f32, kind="ExternalInput").ap()
    D_ap = nc.dram_tensor("D", [T, DH], f32, kind="ExternalInput").ap()
    coD_ap = nc.dram_tensor("co_D", [T, DH], f32, kind="ExternalOutput").ap()
    coS_ap = nc.dram_tensor("co_S", [T, DH], f32, kind="ExternalOutput").ap()

    with tile.TileContext(nc) as tc:
        with (
            tc.tile_pool(name="consts", bufs=1) as consts,
            tc.tile_pool(name="big", bufs=1) as big,
            tc.tile_pool(name="stageD", bufs=p["stageD_bufs"]) as stageD,
            tc.tile_pool(name="rspp", bufs=3) as rspp,
            tc.tile_pool(name="small", bufs=4) as small,
        ):
            ident_f32 = consts.tile([P, P], f32)
            make_identity(nc, ident_f32[:])
            ident_f16 = consts.tile([P, P], f16)
            make_identity(nc, ident_f16[:])
            nbias = consts.tile([P, 1], f32)
            nc.vector.memset(nbias[:], -SHIFT)
            warm_src = consts.tile([P, NTILE], f16)
            nc.vector.memset(warm_src[:], 0.0)

            S_nat = big.tile([P, LT, DH], f16)     # [m%128, (mblk, d)]
            S_T = big.tile([P, KD, T], f16)        # [d%128, (dblk, m)]
            D_nat = big.tile([P, LT, DH], f16)     # [l%128, (lblk, d)]
            W = big.tile([P, LT, T], bf16)         # [l%128, (lblk, m)]
            S1 = big.tile([P, T], f32)             # partial colsums
            nc.vector.memset(S1[:], 0.0)

            gps_ctx = tc.tile_pool(name="gpsum", bufs=p["gpsum_bufs"], space="PSUM")
            gpsum = gps_ctx.__enter__()
            ops_ctx = tc.tile_pool(name="opsum", bufs=p["opsum_bufs"], space="PSUM")
            opsum = ops_ctx.__enter__()
            dtp_ctx = tc.tile_pool(name="dtp", bufs=p["dtp_bufs"])
            dtp = dtp_ctx.__enter__()
            wps_ctx = tc.tile_pool(name="warmps", bufs=1, space="PSUM")
            warmps = wps_ctx.__enter__()
            tps_ctx = tc.tile_pool(name="tps", bufs=2, space="PSUM")
            tps = tps_ctx.__enter__()
            stS_ctx = tc.tile_pool(name="stageS", bufs=p["stageS_bufs"])
            stageS = stS_ctx.__enter__()

            wps = warmps.tile([P, NTILE], f32)

            def _warm(n):
                for _ in range(n):
                    nc.tensor.matmul(wps[:], warm_src[:, 0:P], warm_src[:],
                                     start=True, stop=True)

            def _load_d(i):
                t_ = stageD.tile([P, DH], f32, tag="ldd", name="std")
                nc.gpsimd.dma_start(t_[:], D_ap[i * P:(i + 1) * P, :])
                return t_

            def _conv_d(i, t_):
                nc.scalar.copy(D_nat[:, i, :], t_[:])

            def _mk_dt(i):
                dti = dtp.tile([P, KD, P], f16, tag="dt", name="dt")
                nc.scalar.dma_start(dti[:], D_nat[:, i, :], transpose=True)
                return dti

            def _g_chunk(i, mc, dt_i, rsp):
                gp = gpsum.tile([P, NTILE], f32, tag="g")
                for k in range(KD):
                    nc.tensor.matmul(
                        gp[:],
                        dt_i[:, k, :],
                        S_T[:, k, mc * NTILE:(mc + 1) * NTILE],
                        start=(k == 0),
                        stop=(k == KD - 1),
                    )
                nc.scalar.activation(
                    W[:, i, mc * NTILE:(mc + 1) * NTILE], gp[:], EXP,
                    bias=nbias[:], scale=1.0,
                    accum_out=rsp[:, mc:mc + 1],
                )
                nc.vector.tensor_add(
                    S1[:, mc * NTILE:(mc + 1) * NTILE],
                    S1[:, mc * NTILE:(mc + 1) * NTILE],
                    W[:, i, mc * NTILE:(mc + 1) * NTILE],
                )

            # ---- Prologue ----
            _warm(p["warm_mms"])

            dts = {}
            std_tiles = {0: _load_d(0), 1: _load_d(1)}
            rsps = {0: rspp.tile([P, NCH], f32, tag="rsp", name="rsp0"),
                    1: rspp.tile([P, NCH], f32, tag="rsp", name="rsp1")}

            def _load_s2(t2):
                # one DMA for S blocks 2*t2 and 2*t2+1
                t_ = stageS.tile([P, 2, DH], f32, tag="ld", name="st")
                src_ = S_ap[2 * t2 * P:(2 * t2 + 2) * P, :].rearrange(
                    "(b p) d -> p b d", p=P)
                nc.sync.dma_start(t_[:], src_)
                return t_

            st_tiles = {}
            for t2 in range(p["stageS_bufs"]):
                st_tiles[t2] = _load_s2(t2)

            for j in range(LT):
                t2, bh = j // 2, j % 2
                if bh == 0 and t2 + p["stageS_bufs"] < LT // 2:
                    st_tiles[t2 + p["stageS_bufs"]] = _load_s2(
                        t2 + p["stageS_bufs"])
                stj = st_tiles[t2]
                nc.vector.tensor_copy(S_nat[:, j, :], stj[:, bh, :])
                if bh == 1:
                    st_tiles.pop(t2)
                # S^T for block j via PE transposes (overlaps DMA-bound loads)
                for g in range(2):
                    pt = tps.tile([P, 4, P], f16, tag="tp")
                    for k4 in range(4):
                        k = g * 4 + k4
                        nc.tensor.transpose(
                            pt[:, k4, :], S_nat[:, j, k * P:(k + 1) * P],
                            ident_f16[:],
                        )
                    nc.scalar.copy(
                        S_T[:, g * 4:(g + 1) * 4, j * P:(j + 1) * P], pt[:]
                    )
                _warm(p["keep_warm"])

                # D pipeline milestones + early G chunks
                if j == 1:
                    _conv_d(0, std_tiles.pop(0))
                    std_tiles[2] = _load_d(2)
                elif j == 2:
                    _conv_d(1, std_tiles.pop(1))
                    std_tiles[3] = _load_d(3)
                    dts[0] = _mk_dt(0)
                    dts[1] = _mk_dt(1)
                elif j == 3:
                    _conv_d(2, std_tiles.pop(2))
                elif j == 4:
                    _conv_d(3, std_tiles.pop(3))
                    dts[2] = _mk_dt(2)
                    dts[3] = _mk_dt(3)
                    _g_chunk(0, 0, dts[0], rsps[0])
                    _g_chunk(1, 0, dts[1], rsps[1])
                elif j == 8:
                    _g_chunk(0, 1, dts[0], rsps[0])
                    _g_chunk(1, 1, dts[1], rsps[1])
                elif j == 12:
                    _g_chunk(0, 2, dts[0], rsps[0])
                    _g_chunk(1, 2, dts[1], rsps[1])
            _g_chunk(0, 3, dts[0], rsps[0])
            _g_chunk(1, 3, dts[1], rsps[1])
            dts.pop(0)
            dts.pop(1)

            stS_ctx.__exit__(None, None, None)
            tps_ctx.__exit__(None, None, None)
            wps_ctx.__exit__(None, None, None)

            # ---- Main loop: iter i runs G(i+2) and O_D(i) ----
            outp_ctx = tc.tile_pool(name="outp", bufs=p["outp_bufs"])
            outp = outp_ctx.__enter__()
            wtp_ctx = tc.tile_pool(name="wtp", bufs=p["wtp_bufs"])
            wtp = wtp_ctx.__enter__()
            tpsC_ctx = tc.tile_pool(name="tpsC", bufs=2, space="PSUM")
            tpsC = tpsC_ctx.__enter__()

            def _mk_wt(i):
                wti = wtp.tile([P, LT, P], bf16, tag="wt", name="wt")
                nc.scalar.dma_start(wti[:], W[:, i, :], transpose=True)
                return wti

            # W rows 0/1 are complete; transpose them ahead of the loop
            wts = {0: _mk_wt(0), 1: _mk_wt(1)}

            rcs = None
            cs_p = None
            for i in range(LT):
                # D pipeline: convert block i+3, dt-xbar i+3, load block i+4
                if i + 3 < LT and i + 3 >= 4:
                    _conv_d(i + 3, std_tiles.pop(i + 3))
                    dts[i + 3] = _mk_dt(i + 3)
                if i + 4 < LT:
                    std_tiles[i + 4] = _load_d(i + 4)

                if i + 2 < LT:
                    rsps[i + 2] = rspp.tile([P, NCH], f32, tag="rsp",
                                            name="rspn")
                    dt_i = dts.pop(i + 2)
                    for mc in range(NCH):
                        _g_chunk(i + 2, mc, dt_i, rsps[i + 2])
                    # W row i+2 complete -> issue its W^T xbar now; it
                    # finishes under the next iteration's G matmuls
                    wts[i + 2] = _mk_wt(i + 2)

                # colsum finalize folded into the last two iterations
                if i >= LT - 2:
                    if i == LT - 2:
                        cs_p = small.tile([P, LT], f32, tag="csp")
                    base = (i - (LT - 2)) * 8
                    for jj in range(base, base + 8):
                        ptc = tpsC.tile([P, P], f32, tag="tc")
                        nc.tensor.transpose(
                            ptc[:], S1[:, jj * P:(jj + 1) * P], ident_f32[:]
                        )
                        nc.vector.reduce_sum(cs_p[:, jj:jj + 1], ptc[:], axis=AX)
                    if i == LT - 1:
                        rcs = small.tile([P, LT], f32, tag="rcs")
                        nc.vector.reciprocal(rcs[:], cs_p[:])

                rsp = rsps.pop(i)
                rs = small.tile([P, 1], f32, tag="rs")
                nc.vector.reduce_sum(rs[:], rsp[:], axis=AX)
                rrs = small.tile([P, 1], f32, tag="rrs")
                nc.vector.reciprocal(rrs[:], rs[:])

                wt = wts.pop(i)
                ps = opsum.tile([P, DH], f32, tag="od")
                for kb in range(LT):
                    for n in range(DH // NTILE):
                        nc.tensor.matmul(
                            ps[:, n * NTILE:(n + 1) * NTILE],
                            wt[:, kb, :],
                            S_nat[:, kb, n * NTILE:(n + 1) * NTILE],
                            start=(kb == 0),
                            stop=(kb == LT - 1),
                        )
                o = outp.tile([P, DH], f32, tag="o")
                nc.vector.scalar_tensor_tensor(
                    o[:], ps[:], rrs[:], D_nat[:, i, :], MULT, ADD
                )
                nc.sync.dma_start(coD_ap[i * P:(i + 1) * P, :], o[:])

            tpsC_ctx.__exit__(None, None, None)
            wtp_ctx.__exit__(None, None, None)

            # ---- Phase C: O_S = W.T @ D_nat, emit co_S ----
            opc_ctx = tc.tile_pool(name="opc", bufs=2, space="PSUM")
            opc = opc_ctx.__enter__()
            for j in range(LT):
                ps = opc.tile([P, DH], f32, tag="os")
                o_j = outp.tile([P, DH], f32, tag="o", name="o_j")
                for n in range(2):
                    for lb in range(LT):
                        nc.tensor.matmul(
                            ps[:, n * NTILE:(n + 1) * NTILE],
                            W[:, lb, j * P:(j + 1) * P],
                            D_nat[:, lb, n * NTILE:(n + 1) * NTILE],
                            start=(lb == 0),
                            stop=(lb == LT - 1),
                        )
                    # half n complete: emit it while the other half runs
                    hs = slice(n * NTILE, (n + 1) * NTILE)
                    nc.vector.scalar_tensor_tensor(
                        o_j[:, hs], ps[:, hs], rcs[:, j:j + 1],
                        S_nat[:, j, hs], MULT, ADD,
                    )
                    q = nc.gpsimd if j % 2 == 0 else nc.sync
                    q.dma_start(coS_ap[j * P:(j + 1) * P, hs], o_j[:, hs])
            opc_ctx.__exit__(None, None, None)
            outp_ctx.__exit__(None, None, None)
            dtp_ctx.__exit__(None, None, None)
            ops_ctx.__exit__(None, None, None)
            gps_ctx.__exit__(None, None, None)

    nc.compile()
    return nc


def _get_nc():
    if "nc" not in _CACHE:
        import json as _json
        import os as _o
        ov = _json.loads(_o.environ.get("KOPTS", "{}"))
        _CACHE["nc"] = _build_nc(**ov)
    return _CACHE["nc"]


def kernel(S, D):
    from concourse.bass_utils import run_bass_kernel_spmd

    S = np.ascontiguousarray(np.asarray(S, dtype=np.float32))
    D = np.ascontiguousarray(np.asarray(D, dtype=np.float32))
    B = S.shape[0]
    assert S.shape == (B, T, DH) and D.shape == (B, T, DH) and B == 8

    nc = _get_nc()
    in_maps = [{"S": S[b], "D": D[b]} for b in range(B)]
    res = run_bass_kernel_spmd(nc, in_maps, core_ids=list(range(B)))
    co_D = np.stack([res.results[b]["co_D"] for b in range(B)])
    co_S = np.stack([res.results[b]["co_S"] for b in range(B)])
    return (co_D, co_S)


# revision 10
# speedup vs baseline: 1.2633x; 1.0684x over previous
"""CoAttention kernel v5 for 8 Trainium2 NeuronCores.

Problem: S, D: [8, 2048, 1024] f32, one batch per core.
  G = D @ S^T                      [2048, 2048]
  co_D = D + rowsoftmax(G) @ S
  co_S = S + rowsoftmax(G^T) @ D

Design notes (v5):
 - S^T is built by PE transposes interleaved with the DMA-bound S load
   stream in the prologue (xbar-transposing it there floods the shared
   DMA-semaphore pool and serializes the loads -- 100us regression).
 - D^T tiles (dt) and W^T tiles (wt) come from DMA-xbar transposes
   (dma_start(transpose=True), ~1.3/1.9us each) issued in the main loop
   where DMA traffic is sparse; wt_i is issued one iteration before its
   consumer so xbar latency hides under G matmuls.
 - A dummy-MM warmup burst plus one keep-warm dummy per S block holds
   the PE HAM clock gate at 2.4 GHz through the prologue (PE transposes
   do not count as HAM-busy; v2 ran its first 27us at 1.2 GHz).
 - Pools are scoped: the 5-deep S stage pool closes after the prologue
   (5-deep is needed to hide per-hop semaphore latency; 2-deep ran the
   load stream at 1/3 speed), freeing SBUF for the wt/outp pools.
 - colsum finalize (16 PE f32 transposes) folded into iters 14/15;
   phase C emits co_S per half, stores split across two queues.
 - Residual adds use the resident f16 S_nat/D_nat (~2e-4 extra rel
   err), so phase C reloads nothing from DRAM.

Softmax trick (v2): shift-invariance with constant SHIFT; shared
W = exp(G - SHIFT) bf16 serves both directions:
  co_D[l] = D[l] + (W @ S)[l] / rowsum_l(W)
  co_S[m] = S[m] + (W^T @ D)[m] / colsum_m(W)
"""

import numpy as np

P = 128
T = 2048
DH = 1024
LT = T // P     # 16 token blocks per side
KD = DH // P    # 8 contraction blocks
NTILE = 512
NCH = T // NTILE  # 4 chunks of the m axis
SHIFT = 100.0

DEFAULTS = dict(
    warm_mms=12,
    keep_warm=1,
    stageS_bufs=5,
    stageD_bufs=2,
    gpsum_bufs=2,
    opsum_bufs=1,
    dtp_bufs=4,
    wtp_bufs=3,
    outp_bufs=2,
)

_CACHE = {}


def _build_nc(**overrides):
    import concourse.mybir as mybir
    import concourse.tile as tile
    from concourse import bacc
    from concourse.masks import make_identity

    p = dict(DEFAULTS)
    p.update(overrides)

    dt = mybir.dt
    f32, f16, bf16 = dt.float32, dt.float16, dt.bfloat16
    AX = mybir.AxisListType.X
    EXP = mybir.ActivationFunctionType.Exp
    MULT = mybir.AluOpType.mult
    ADD = mybir.AluOpType.add

    nc = bacc.Bacc("TRN2", target_bir_lowering=False, debug=False)

    S_ap = nc.dram_tensor("S", [T, DH], f32, kind="ExternalInput").ap()
    D_ap = nc.dram_tensor("D", [T, DH], f32, kind="ExternalInput").ap()
    coD_ap = nc.dram_tensor("co_D", [T, DH], f32, kind="ExternalOutput").ap()
    coS_ap = nc.dram_tensor("co_S", [T, DH], f32, kind="ExternalOutput").ap()

    with tile.TileContext(nc) as tc:
        with (
            tc.tile_pool(name="consts", bufs=1) as consts,
            tc.tile_pool(name="big", bufs=1) as big,
            tc.tile_pool(name="stageD", bufs=p["stageD_bufs"]) as stageD,
            tc.tile_pool(name="rspp", bufs=3) as rspp,
            tc.tile_pool(name="small", bufs=4) as small,
        ):
            ident_f32 = consts.tile([P, P], f32)
            make_identity(nc, ident_f32[:])
            ident_f16 = consts.tile([P, P], f16)
            make_identity(nc, ident_f16[:])
            nbias = consts.tile([P, 1], f32)
            nc.vector.memset(nbias[:], -SHIFT)
            warm_src = consts.tile([P, NTILE], f16)
            nc.vector.memset(warm_src[:], 0.0)

            S_nat = big.tile([P, LT, DH], f16)     # [m%128, (mblk, d)]
            S_T = big.tile([P, KD, T], f16)        # [d%128, (dblk, m)]
            D_nat = big.tile([P, LT, DH], f16)     # [l%128, (lblk, d)]
            W = big.tile([P, LT, T], bf16)         # [l%128, (lblk, m)]
            S1 = big.tile([P, T], f32)             # partial colsums
            nc.vector.memset(S1[:], 0.0)

            gps_ctx = tc.tile_pool(name="gpsum", bufs=p["gpsum_bufs"], space="PSUM")
            gpsum = gps_ctx.__enter__()
            ops_ctx = tc.tile_pool(name="opsum", bufs=p["opsum_bufs"], space="PSUM")
            opsum = ops_ctx.__enter__()
            dtp_ctx = tc.tile_pool(name="dtp", bufs=p["dtp_bufs"])
            dtp = dtp_ctx.__enter__()
            wps_ctx = tc.tile_pool(name="warmps", bufs=1, space="PSUM")
            warmps = wps_ctx.__enter__()
            tps_ctx = tc.tile_pool(name="tps", bufs=2, space="PSUM")
            tps = tps_ctx.__enter__()
            stS_ctx = tc.tile_pool(name="stageS", bufs=p["stageS_bufs"])
            stageS = stS_ctx.__enter__()

            wps = warmps.tile([P, NTILE], f32)

            def _warm(n):
                for _ in range(n):
                    nc.tensor.matmul(wps[:], warm_src[:, 0:P], warm_src[:],
                                     start=True, stop=True)

            def _load_d(i):
                t_ = stageD.tile([P, DH], f32, tag="ldd", name="std")
                nc.gpsimd.dma_start(t_[:], D_ap[i * P:(i + 1) * P, :])
                return t_

            def _conv_d(i, t_):
                nc.scalar.copy(D_nat[:, i, :], t_[:])

            def _mk_dt(i):
                dti = dtp.tile([P, KD, P], f16, tag="dt", name="dt")
                nc.scalar.dma_start(dti[:], D_nat[:, i, :], transpose=True)
                return dti

            def _g_chunk(i, mc, dt_i, rsp):
                gp = gpsum.tile([P, NTILE], f32, tag="g")
                for k in range(KD):
                    nc.tensor.matmul(
                        gp[:],
                        dt_i[:, k, :],
                        S_T[:, k, mc * NTILE:(mc + 1) * NTILE],
                        start=(k == 0),
                        stop=(k == KD - 1),
                    )
                nc.scalar.activation(
                    W[:, i, mc * NTILE:(mc + 1) * NTILE], gp[:], EXP,
                    bias=nbias[:], scale=1.0,
                    accum_out=rsp[:, mc:mc + 1],
                )
                nc.vector.tensor_add(
                    S1[:, mc * NTILE:(mc + 1) * NTILE],
                    S1[:, mc * NTILE:(mc + 1) * NTILE],
                    W[:, i, mc * NTILE:(mc + 1) * NTILE],
                )

            def _mk_dt_pe(i):
                # D^T tiles via PE transposes (prologue only: no xbar DMAs
                # near the load stream -- they poison the DMA sem pool)
                dti = dtp.tile([P, KD, P], f16, tag="dt", name="dt")
                for g in range(2):
                    pt = tps.tile([P, 4, P], f16, tag="tp")
                    for k4 in range(4):
                        k = g * 4 + k4
                        nc.tensor.transpose(
                            pt[:, k4, :], D_nat[:, i, k * P:(k + 1) * P],
                            ident_f16[:],
                        )
                    nc.scalar.copy(dti[:, g * 4:(g + 1) * 4, :], pt[:])
                return dti

            # ---- Prologue ----
            _warm(p["warm_mms"])

            dts = {}
            std_tiles = {0: _load_d(0), 1: _load_d(1)}
            rsps = {0: rspp.tile([P, NCH], f32, tag="rsp", name="rsp0"),
                    1: rspp.tile([P, NCH], f32, tag="rsp", name="rsp1")}

            st_tiles = {}
            for j in range(p["stageS_bufs"]):
                st_tiles[j] = stageS.tile([P, DH], f32, tag="ld", name="st")
                nc.sync.dma_start(st_tiles[j][:], S_ap[j * P:(j + 1) * P, :])

            for j in range(LT):
                if j + p["stageS_bufs"] < LT:
                    k = j + p["stageS_bufs"]
                    st_tiles[k] = stageS.tile([P, DH], f32, tag="ld", name="st")
                    nc.sync.dma_start(st_tiles[k][:], S_ap[k * P:(k + 1) * P, :])
                stj = st_tiles.pop(j)
                nc.vector.tensor_copy(S_nat[:, j, :], stj[:])
                # S^T for block j via PE transposes (overlaps DMA-bound loads)
                for g in range(2):
                    pt = tps.tile([P, 4, P], f16, tag="tp")
                    for k4 in range(4):
                        k = g * 4 + k4
                        nc.tensor.transpose(
                            pt[:, k4, :], S_nat[:, j, k * P:(k + 1) * P],
                            ident_f16[:],
                        )
                    nc.scalar.copy(
                        S_T[:, g * 4:(g + 1) * 4, j * P:(j + 1) * P], pt[:]
                    )
                _warm(p["keep_warm"])

                # D pipeline milestones + early G chunks
                if j == 1:
                    _conv_d(0, std_tiles.pop(0))
                    std_tiles[2] = _load_d(2)
                elif j == 2:
                    _conv_d(1, std_tiles.pop(1))
                    std_tiles[3] = _load_d(3)
                    dts[0] = _mk_dt_pe(0)
                elif j == 3:
                    _conv_d(2, std_tiles.pop(2))
                    dts[1] = _mk_dt_pe(1)
                elif j == 4:
                    _conv_d(3, std_tiles.pop(3))
                    _g_chunk(0, 0, dts[0], rsps[0])
                    _g_chunk(1, 0, dts[1], rsps[1])
                elif j == 5:
                    dts[2] = _mk_dt_pe(2)
                elif j == 6:
                    dts[3] = _mk_dt_pe(3)
                elif j == 8:
                    _g_chunk(0, 1, dts[0], rsps[0])
                    _g_chunk(1, 1, dts[1], rsps[1])
                elif j == 12:
                    _g_chunk(0, 2, dts[0], rsps[0])
                    _g_chunk(1, 2, dts[1], rsps[1])
            _g_chunk(0, 3, dts[0], rsps[0])
            _g_chunk(1, 3, dts[1], rsps[1])
            dts.pop(0)
            dts.pop(1)

            stS_ctx.__exit__(None, None, None)
            tps_ctx.__exit__(None, None, None)
            wps_ctx.__exit__(None, None, None)

            # ---- Main loop: iter i runs G(i+2) and O_D(i) ----
            outp_ctx = tc.tile_pool(name="outp", bufs=p["outp_bufs"])
            outp = outp_ctx.__enter__()
            wtp_ctx = tc.tile_pool(name="wtp", bufs=p["wtp_bufs"])
            wtp = wtp_ctx.__enter__()
            tpsC_ctx = tc.tile_pool(name="tpsC", bufs=2, space="PSUM")
            tpsC = tpsC_ctx.__enter__()

            def _mk_wt(i):
                wti = wtp.tile([P, LT, P], bf16, tag="wt", name="wt")
                nc.scalar.dma_start(wti[:], W[:, i, :], transpose=True)
                return wti

            # W rows 0/1 are complete; transpose them ahead of the loop
            wts = {0: _mk_wt(0), 1: _mk_wt(1)}

            rcs = None
            cs_p = None
            for i in range(LT):
                # D pipeline: convert block i+3, dt-xbar i+3, load block i+4
                if i + 3 < LT and i + 3 >= 4:
                    _conv_d(i + 3, std_tiles.pop(i + 3))
                    dts[i + 3] = _mk_dt(i + 3)
                if i + 4 < LT:
                    std_tiles[i + 4] = _load_d(i + 4)

                if i + 2 < LT:
                    rsps[i + 2] = rspp.tile([P, NCH], f32, tag="rsp",
                                            name="rspn")
                    dt_i = dts.pop(i + 2)
                    for mc in range(NCH):
                        _g_chunk(i + 2, mc, dt_i, rsps[i + 2])
                    # W row i+2 complete -> issue its W^T xbar now; it
                    # finishes under the next iteration's G matmuls
                    wts[i + 2] = _mk_wt(i + 2)

                # colsum finalize folded into the last two iterations
                if i >= LT - 2:
                    if i == LT - 2:
                        cs_p = small.tile([P, LT], f32, tag="csp")
                    base = (i - (LT - 2)) * 8
                    for jj in range(base, base + 8):
                        ptc = tpsC.tile([P, P], f32, tag="tc")
                        nc.tensor.transpose(
                            ptc[:], S1[:, jj * P:(jj + 1) * P], ident_f32[:]
                        )
                        nc.vector.reduce_sum(cs_p[:, jj:jj + 1], ptc[:], axis=AX)
                    if i == LT - 1:
                        rcs = small.tile([P, LT], f32, tag="rcs")
                        nc.vector.reciprocal(rcs[:], cs_p[:])

                rsp = rsps.pop(i)
                rs = small.tile([P, 1], f32, tag="rs")
                nc.vector.reduce_sum(rs[:], rsp[:], axis=AX)
                rrs = small.tile([P, 1], f32, tag="rrs")
                nc.vector.reciprocal(rrs[:], rs[:])

                wt = wts.pop(i)
                ps = opsum.tile([P, DH], f32, tag="od")
                for kb in range(LT):
                    for n in range(DH // NTILE):
                        nc.tensor.matmul(
                            ps[:, n * NTILE:(n + 1) * NTILE],
                            wt[:, kb, :],
                            S_nat[:, kb, n * NTILE:(n + 1) * NTILE],
                            start=(kb == 0),
                            stop=(kb == LT - 1),
                        )
                o = outp.tile([P, DH], f32, tag="o")
                nc.vector.scalar_tensor_tensor(
                    o[:], ps[:], rrs[:], D_nat[:, i, :], MULT, ADD
                )
                nc.sync.dma_start(coD_ap[i * P:(i + 1) * P, :], o[:])

            tpsC_ctx.__exit__(None, None, None)
            wtp_ctx.__exit__(None, None, None)

            # ---- Phase C: O_S = W.T @ D_nat, emit co_S ----
            opc_ctx = tc.tile_pool(name="opc", bufs=2, space="PSUM")
            opc = opc_ctx.__enter__()
            for j in range(LT):
                ps = opc.tile([P, DH], f32, tag="os")
                o_j = outp.tile([P, DH], f32, tag="o", name="o_j")
                for n in range(2):
                    for lb in range(LT):
                        nc.tensor.matmul(
                            ps[:, n * NTILE:(n + 1) * NTILE],
                            W[:, lb, j * P:(j + 1) * P],
                            D_nat[:, lb, n * NTILE:(n + 1) * NTILE],
                            start=(lb == 0),
                            stop=(lb == LT - 1),
                        )
                    # half n complete: emit it while the other half runs
                    hs = slice(n * NTILE, (n + 1) * NTILE)
                    nc.vector.scalar_tensor_tensor(
                        o_j[:, hs], ps[:, hs], rcs[:, j:j + 1],
                        S_nat[:, j, hs], MULT, ADD,
                    )
                    q = nc.gpsimd if j % 2 == 0 else nc.sync
                    q.dma_start(coS_ap[j * P:(j + 1) * P, hs], o_j[:, hs])
            opc_ctx.__exit__(None, None, None)
            outp_ctx.__exit__(None, None, None)
            dtp_ctx.__exit__(None, None, None)
            ops_ctx.__exit__(None, None, None)
            gps_ctx.__exit__(None, None, None)

    nc.compile()
    return nc


def _get_nc():
    if "nc" not in _CACHE:
        import json as _json
        import os as _o
        ov = _json.loads(_o.environ.get("KOPTS", "{}"))
        _CACHE["nc"] = _build_nc(**ov)
    return _CACHE["nc"]


def kernel(S, D):
    from concourse.bass_utils import run_bass_kernel_spmd

    S = np.ascontiguousarray(np.asarray(S, dtype=np.float32))
    D = np.ascontiguousarray(np.asarray(D, dtype=np.float32))
    B = S.shape[0]
    assert S.shape == (B, T, DH) and D.shape == (B, T, DH) and B == 8

    nc = _get_nc()
    in_maps = [{"S": S[b], "D": D[b]} for b in range(B)]
    res = run_bass_kernel_spmd(nc, in_maps, core_ids=list(range(B)))
    co_D = np.stack([res.results[b]["co_D"] for b in range(B)])
    co_S = np.stack([res.results[b]["co_S"] for b in range(B)])
    return (co_D, co_S)
